# revision 1
# baseline (speedup 1.0000x reference)
"""Trainium2 Bass kernel for CodeAttention (B=4, S=2048, E=768, H=12).

Sharding: 8 cores = 4 batches x 2 head-groups (6 heads each).
Each core computes a partial projection output for its batch; the host
sums the two partials per batch and adds the (host-folded) bias row.
"""

import sys

if "/opt/trn_rl_repo" not in sys.path:
    sys.path.insert(0, "/opt/trn_rl_repo")

import numpy as np

import concourse.bass as bass  # noqa: F401  (engine types referenced via nc)
import concourse.mybir as mybir
import concourse.tile as tile
from concourse import bacc
from concourse.alu_op_type import AluOpType
from concourse.bass_utils import run_bass_kernel_spmd
from concourse.masks import make_identity

F32 = mybir.dt.float32
F32R = mybir.dt.float32r
Act = mybir.ActivationFunctionType

B, S, E, H, D = 4, 2048, 768, 12, 64
HC = 6                    # heads per core
QKC = HC * D * 2          # qk columns per core = 768
VC = HC * D               # v columns per core = 384
KCH = E // 128            # contraction chunks over E = 6
NKC = S // 128            # key chunks = 16
NQB = S // 512            # q blocks of 512 = 4
NSB = S // 512            # s blocks of 512 = 4
VW = D + 1                # v width incl. ones column = 65
MASK_NEG = -50.0


def build_program():
    nc = bacc.Bacc("TRN2", target_bir_lowering=False, debug=False, num_devices=8)

    x_d = nc.dram_tensor("x", [S, E], F32, kind="ExternalInput")
    wqk_d = nc.dram_tensor("wqk", [KCH, 128, QKC], F32R, kind="ExternalInput")
    wv_d = nc.dram_tensor("wv", [KCH, 128, VC], F32R, kind="ExternalInput")
    wp_d = nc.dram_tensor("wp", [VC // 128, 128, E], F32R, kind="ExternalInput")
    bqk_d = nc.dram_tensor("bqk", [QKC], F32, kind="ExternalInput")
    mb_d = nc.dram_tensor("mb", [S], F32, kind="ExternalInput")
    y_d = nc.dram_tensor("y", [S, E], F32, kind="ExternalOutput")

    with tile.TileContext(nc) as tc:
        _emit(nc, tc, x_d, wqk_d, wv_d, wp_d, bqk_d, mb_d, y_d)
    nc.compile()
    return nc


def _emit(nc, tc, x_d, wqk_d, wv_d, wp_d, bqk_d, mb_d, y_d):
    ctx_pools = []

    def pool(name, bufs, space="SBUF"):
        p = tc.tile_pool(name=name, bufs=bufs, space=space)
        ctx_pools.append(p)
        return p.__enter__()

    consts = pool("consts", 1)
    store = pool("store", 1)

    ident = consts.tile([128, 128], F32)
    make_identity(nc, ident[:])
    ones_row_f = consts.tile([1, D], F32)
    nc.vector.memset(ones_row_f[:], 1.0)
    ones_row = consts.tile([1, D], F32R)
    nc.vector.tensor_copy(ones_row[:], ones_row_f[:])

    # weights go over the SWDGE (gpsimd) queue so the x-chunk loads on the
    # sync HWDGE queue aren't serialized behind 4.7MB of weight traffic.
    wqk = consts.tile([128, KCH, QKC], F32R)
    wv = consts.tile([128, KCH, VC], F32R)
    wp = consts.tile([128, VC // 128, E], F32R)
    for k in range(KCH):
        nc.gpsimd.dma_start(wv[:, k, :], wv_d.ap()[k])
    for k in range(KCH):
        nc.gpsimd.dma_start(wqk[:, k, :], wqk_d.ap()[k])
    for t in range(VC // 128):
        nc.gpsimd.dma_start(wp[:, t, :], wp_d.ap()[t])

    bqk = consts.tile([128, QKC // 128], F32)
    nc.scalar.dma_start(bqk[:], bqk_d.ap().rearrange("(c p) -> p c", p=128))
    mb = consts.tile([128, NKC], F32)
    nc.scalar.dma_start(mb[:], mb_d.ap().rearrange("(c p) -> p c", p=128))

    # qkT store, one tile per s-block so attention deps are per-block:
    # tile m of 6 holds W-columns m*128..; q cols 0..383, k cols 384..767.
    qkT = [
        store.tile([128, QKC // 128, 512], F32R, name=f"qkT{sb}")
        for sb in range(NSB)
    ]
    # v store: per s-block [s-chunk, head, 65] with ones in column 64.
    vst = [
        store.tile([128, 4, HC, VW], F32R, name=f"vst{sb}") for sb in range(NSB)
    ]
    ones_f = consts.tile([128, 4 * HC], F32)
    nc.vector.memset(ones_f[:], 1.0)
    for sb in range(NSB):
        nc.vector.tensor_copy(
            vst[sb][:, :, :, D : D + 1],
            ones_f[:].rearrange("p (a b one) -> p a b one", a=4, b=HC, one=1),
        )
    # attn output (transposed): tile t rows = head dims 2t,2t+1.
    att = store.tile([128, VC // 128, S], F32R)

    # ---- Phase 1: QKV projections ----
    with (
        tc.tile_pool(name="xs", bufs=3) as xs_p,
        tc.tile_pool(name="xt", bufs=3) as xt_p,
        tc.tile_pool(name="tp", bufs=3, space="PSUM") as tp_p,
        tc.tile_pool(name="va", bufs=2, space="PSUM") as va_p,
        tc.tile_pool(name="qk", bufs=3, space="PSUM") as qk_p,
    ):
        _emit_qkv(nc, x_d, ident, wqk, wv, bqk, qkT, vst, xs_p, xt_p, tp_p, va_p, qk_p)

    # ---- Phase 2: attention + projection ----
    st_p = pool("st", 2, space="PSUM")       # [128,1024] = 2 banks each
    pv_p = pool("pv", 3, space="PSUM")
    misc_p = pool("miscp", 1, space="PSUM")  # shared bc/ya slot
    pt_p = pool("pt", 3)
    se_p = pool("se", 2)
    rb_p = pool("rb", 2)
    ys_p = pool("ys", 2)

    for qb in range(NQB):
        qs = slice(qb * 512, (qb + 1) * 512)
        deferred_norm = None
        for hp in range(HC // 2):
            pvs = [
                pv_p.tile([128, 512], F32, tag="pv", name=f"pv{qb}_{hp}_{i}")
                for i in range(2)
            ]
            for kc in range(NKC):
                # both heads of the pair share one 2-bank score tile so a
                # single (cheaper) exp covers them: free dim 1024 amortizes
                # ACT's per-instruction overhead.
                st = st_p.tile([128, 1024], F32, tag="st")
                for sub in range(2):
                    r0 = sub * 64
                    kb, ko = kc // 4, kc % 4
                    nc.tensor.matmul(
                        st[:, sub * 512 : (sub + 1) * 512],
                        qkT[kb][r0 : r0 + 64, 3 + hp, ko * 128 : (ko + 1) * 128],
                        qkT[qb][r0 : r0 + 64, hp, :],
                        start=True, stop=True,
                    )
                pt = pt_p.tile([128, 1024], F32R, tag="pt")
                nc.scalar.activation(
                    pt[:], st[:], Act.Exp, bias=mb[:, kc : kc + 1], scale=0.125
                )
                for sub in range(2):
                    h = hp * 2 + sub
                    nc.tensor.matmul(
                        pvs[sub][0:VW, :], vst[kc // 4][:, kc % 4, h, :],
                        pt[:, sub * 512 : (sub + 1) * 512],
                        start=(kc == 0), stop=(kc == NKC - 1),
                    )
                if kc == 1 and deferred_norm is not None:
                    deferred_norm()
                    deferred_norm = None
            def norm_pair(pvs=pvs, hp=hp, qs=qs):
                for sub in range(2):
                    se = se_p.tile([1, 512], F32R, tag="se", name="se")
                    nc.vector.tensor_copy(se[:], pvs[sub][D : D + 1, :])
                    bc = misc_p.tile([128, 512], F32, tag="miscp", name="bc")
                    nc.tensor.matmul(
                        bc[0:D, :], ones_row[:], se[:], start=True, stop=True
                    )
                    rb = rb_p.tile([D, 512], F32R, tag="rb", name="rb")
                    with nc.allow_low_precision(reason="f32r is full width"):
                        nc.vector.reciprocal(rb[:], bc[0:D, :])
                    nc.vector.tensor_tensor(
                        att[sub * 64 : sub * 64 + 64, hp, qs],
                        pvs[sub][0:D, :], rb[:], op=AluOpType.mult,
                    )
            deferred_norm = norm_pair
        if deferred_norm is not None:
            deferred_norm()
            deferred_norm = None
        # projection for this q-block
        for sc in range(4):
            sg = qb * 4 + sc
            ys = ys_p.tile([128, E], F32, tag="ys")
            for n0, nw in ((0, 512), (512, 256)):
                if qb == NQB - 1:
                    # attention done; reuse idle score-pool banks so the
                    # final projection isn't serialized on one slot
                    ya = st_p.tile([128, 512], F32, tag="st", name="ya")
                else:
                    ya = misc_p.tile([128, 512], F32, tag="miscp")
                for t in range(VC // 128):
                    nc.tensor.matmul(
                        ya[:, :nw],
                        att[:, t, sg * 128 : (sg + 1) * 128],
                        wp[:, t, n0 : n0 + nw],
                        start=(t == 0), stop=(t == VC // 128 - 1),
                    )
                nc.vector.tensor_copy(ys[:, n0 : n0 + nw], ya[:, :nw])
            nc.sync.dma_start(y_d.ap()[sg * 128 : (sg + 1) * 128, :], ys[:])

    for p in reversed(ctx_pools):
        p.__exit__(None, None, None)


def _emit_qkv(nc, x_d, ident, wqk, wv, bqk, qkT, vst, xs_p, xt_p, tp_p, va_p, qk_p):
    for sb in range(NSB):
        xt = xt_p.tile([128, KCH, 512], F32R)
        for sc in range(4):
            sg = sb * 4 + sc
            xs = xs_p.tile([128, E], F32)
            nc.sync.dma_start(xs[:], x_d.ap()[sg * 128 : (sg + 1) * 128, :])
            # batch 4 transposes per PSUM bank, then 3 -> one DVE copy each
            for g in range(2):
                kn = 4 if g == 0 else 2
                tp = tp_p.tile([128, 512], F32, tag="tp")
                for kk in range(kn):
                    k = g * 4 + kk
                    # 4 transposes share one PSUM bank as one accumulation
                    # group (disjoint columns, per-element has_written).
                    nc.tensor.matmul(
                        tp[:, kk * 128 : (kk + 1) * 128],
                        xs[:, k * 128 : (k + 1) * 128], ident[:],
                        is_transpose=True,
                        start=(kk == 0), stop=(kk == kn - 1),
                    )
                nc.vector.tensor_copy(
                    xt[:, g * 4 : g * 4 + kn, sc * 128 : (sc + 1) * 128],
                    tp[:, : kn * 128].rearrange("p (k f) -> p k f", k=kn),
                )
            va = va_p.tile([128, VC], F32)
            for k in range(KCH):
                nc.tensor.matmul(
                    va[:], xt[:, k, sc * 128 : (sc + 1) * 128], wv[:, k, :],
                    start=(k == 0), stop=(k == KCH - 1),
                )
            nc.vector.tensor_copy(
                vst[sb][:, sc, :, 0:D],
                va[:].rearrange("p (h d) -> p h d", h=HC),
            )
        for m in range(QKC // 128):
            qk = qk_p.tile([128, 512], F32)
            for k in range(KCH):
                nc.tensor.matmul(
                    qk[:], wqk[:, k, m * 128 : (m + 1) * 128], xt[:, k, :],
                    start=(k == 0), stop=(k == KCH - 1),
                )
            nc.vector.tensor_scalar_add(
                qkT[sb][:, m, :], qk[:], bqk[:, m : m + 1]
            )


def make_core_inputs(x, mask, Wqkv, bqkv):
    """Slice full inputs into 8 per-core input maps."""
    x = np.ascontiguousarray(np.asarray(x, dtype=np.float32))
    mask = np.asarray(mask)
    Wqkv = np.asarray(Wqkv, dtype=np.float32)
    bqkv = np.asarray(bqkv, dtype=np.float32)
    in_maps = []
    for c in range(8):
        b = c // 2
        h0 = (c % 2) * HC
        wq = Wqkv[:, h0 * D : (h0 + HC) * D]
        wk = Wqkv[:, E + h0 * D : E + (h0 + HC) * D]
        wqk = np.concatenate([wq, wk], axis=1).reshape(KCH, 128, QKC)
        wv = Wqkv[:, 2 * E + h0 * D : 2 * E + (h0 + HC) * D].reshape(KCH, 128, VC)
        bqk = np.concatenate(
            [bqkv[h0 * D : (h0 + HC) * D], bqkv[E + h0 * D : E + (h0 + HC) * D]]
        )
        mb = np.where(mask[b, 0, 0, :] == 0, np.float32(MASK_NEG), np.float32(0.0))
        in_maps.append(
            {
                "x": np.ascontiguousarray(x[b]),
                "wqk": np.ascontiguousarray(wqk),
                "wv": np.ascontiguousarray(wv),
                "wp": None,  # filled below (needs Wproj)
                "bqk": np.ascontiguousarray(bqk.astype(np.float32)),
                "mb": np.ascontiguousarray(mb.astype(np.float32)),
            }
        )
    return in_maps


def run(x, mask, Wqkv, bqkv, Wproj, bproj, trace=False, trace_cores=None):
    Wproj = np.asarray(Wproj, dtype=np.float32)
    bproj = np.asarray(bproj, dtype=np.float32)
    bqkv_np = np.asarray(bqkv, dtype=np.float32)
    in_maps = make_core_inputs(x, mask, Wqkv, bqkv_np)
    for c in range(8):
        h0 = (c % 2) * HC
        wp = Wproj[h0 * D : (h0 + HC) * D, :].reshape(VC // 128, 128, E)
        in_maps[c]["wp"] = np.ascontiguousarray(wp)

    nc = build_program()
    try:
        res = run_bass_kernel_spmd(
            nc, in_maps, core_ids=list(range(8)), trace=trace,
            trace_cores=trace_cores,
        )
    except Exception:
        # transient device wedge (e.g. NRT_EXEC_UNIT_UNRECOVERABLE) —
        # one retry is usually enough
        res = run_bass_kernel_spmd(
            nc, in_maps, core_ids=list(range(8)), trace=trace,
            trace_cores=trace_cores,
        )
    parts = [res.results[c]["y"] for c in range(8)]

    # host-folded bias: v-bias passes through softmax (weights sum to 1),
    # so y += bv @ Wproj + bproj, applied once per batch row.
    bv = bqkv_np[2 * E : 3 * E]
    bias_row = bv @ Wproj + bproj
    y = np.stack(
        [parts[2 * b] + parts[2 * b + 1] + bias_row for b in range(B)]
    ).astype(np.float32)
    return y, res


def kernel(x, mask, Wqkv, bqkv, Wproj, bproj):
    y, _ = run(x, mask, Wqkv, bqkv, Wproj, bproj, trace=False)
    return y



# revision 19
# speedup vs baseline: 1.4585x; 1.4585x over previous
"""Trainium2 Bass kernel for CodeAttention (B=4, S=2048, E=768, H=12).

Sharding: 8 cores = 4 batches x 2 head-groups (6 heads each); each core
computes a partial projection output for its batch, host sums the two
partials and adds the (host-folded) v-bias/proj-bias row.

Key optimizations over the naive formulation:
 - masked keys (mask==0, ~50%) are compacted away on the host: K/V are
   computed only for unmasked key positions (exact: reference's -1e9
   additive mask makes masked keys contribute exactly 0).
 - x / weights travel as bf16; x is transposed by the DMA xbar
   (dma_start_transpose) instead of PE identity-matmuls.
 - optional fp8e4 DoubleRow matmuls (2 contraction groups per pass) for
   the score, attn*V and output-projection stages.
"""

import sys

if "/opt/trn_rl_repo" not in sys.path:
    sys.path.insert(0, "/opt/trn_rl_repo")

import ml_dtypes
import numpy as np

import concourse.bass as bass  # noqa: F401
import concourse.mybir as mybir
import concourse.tile as tile
from concourse import bacc
from concourse.alu_op_type import AluOpType
from concourse.bass_utils import run_bass_kernel_spmd

F32 = mybir.dt.float32
F32R = mybir.dt.float32r
BF16 = mybir.dt.bfloat16
FP8 = mybir.dt.float8e4
Act = mybir.ActivationFunctionType
DR = mybir.MatmulPerfMode.DoubleRow

B, S, E, H, D = 4, 2048, 768, 12, 64
HC = 6                    # heads per core
KCH = E // 128            # x-feature chunks = 6
VC = HC * D               # v columns per core = 384
VW = D + 1                # v width incl. ones column = 65
NQB = S // 512            # q blocks = 4
MASK_NEG = -50.0

# fp8 DoubleRow stage flags (host + device layouts derive from these)
SC_FP8 = False            # scores q@k in fp8e4 (+ lo/hi W-column layout)
PV_FP8 = False            # attn*V in fp8e4 (pt + vst)
OP_FP8 = False            # out-proj in fp8e4 (att scaled x16, wp/16)

QK_DT = FP8 if SC_FP8 else BF16
PV_DT = FP8 if PV_FP8 else BF16
ATT_DT = FP8 if OP_FP8 else BF16
ATT_SCALE = 16.0 if OP_FP8 else 1.0

# defaults for build_program() without a preceding run() (TimelineSim in
# test.py); overwritten by run() with the values for the actual inputs.
_BUILD = {"skv": 1152, "n_min": 986}


def np_dt(dt):
    return mybir.dt.np(dt)


def build_program(skv=None, n_min=None, debug=False):
    skv = skv if skv is not None else _BUILD["skv"]
    n_min = n_min if n_min is not None else _BUILD["n_min"]
    nc = bacc.Bacc("TRN2", target_bir_lowering=False, debug=False, num_devices=8)

    xq_d = nc.dram_tensor("xq", [S, E], BF16, kind="ExternalInput")
    xkv_d = nc.dram_tensor("xkv", [skv, E], BF16, kind="ExternalInput")
    # weights pre-transposed on host to partition-major [128, KCH*cols]
    wq_d = nc.dram_tensor("wq", [128, KCH * VC], QK_DT, kind="ExternalInput")
    wk_d = nc.dram_tensor("wk", [128, KCH * VC], QK_DT, kind="ExternalInput")
    wv_d = nc.dram_tensor("wv", [128, KCH * VC], BF16, kind="ExternalInput")
    wp_d = nc.dram_tensor("wp", [128, (VC // 128) * E], ATT_DT, kind="ExternalInput")
    bq_d = nc.dram_tensor("bq", [VC], F32, kind="ExternalInput")
    bk_d = nc.dram_tensor("bk", [VC], F32, kind="ExternalInput")
    mb_d = nc.dram_tensor("mb", [skv], F32, kind="ExternalInput")
    y_d = nc.dram_tensor("y", [S, E], BF16, kind="ExternalOutput")
    dbg = None
    if debug:
        dbg = {
            "qT": nc.dram_tensor("dbg_qT", [128, 3 * S], QK_DT, kind="ExternalOutput"),
            "kT": nc.dram_tensor("dbg_kT", [128, 3 * skv], QK_DT, kind="ExternalOutput"),
            "vst": nc.dram_tensor("dbg_vst", [128, (skv // 128) * HC * VW], PV_DT, kind="ExternalOutput"),
            "att": nc.dram_tensor("dbg_att", [128, 3 * S], ATT_DT, kind="ExternalOutput"),
            "xqT": nc.dram_tensor("dbg_xqT", [128, KCH * S], BF16, kind="ExternalOutput"),
            "xkvT": nc.dram_tensor("dbg_xkvT", [128, KCH * skv], BF16, kind="ExternalOutput"),
        }

    with tile.TileContext(nc) as tc:
        _emit(nc, tc, skv, n_min,
              xq_d, xkv_d, wq_d, wk_d, wv_d, wp_d, bq_d, bk_d, mb_d, y_d, dbg)
    nc.compile()
    return nc


def _score_slices(t, h, kc_or_q):
    """(sliced AP, DR?) for head h in the qT/kT layout."""
    if SC_FP8 and h < 4:
        return t[32 * h : 32 * h + 32, 0:2, kc_or_q], True
    if SC_FP8:
        return t[64 * (h - 4) : 64 * (h - 4) + 64, 2, kc_or_q], False
    return t[64 * (h % 2) : 64 * (h % 2) + 64, h // 2, kc_or_q], False


def _emit(nc, tc, skv, n_min,
          xq_d, xkv_d, wq_d, wk_d, wv_d, wp_d, bq_d, bk_d, mb_d, y_d, dbg=None):
    nkc = skv // 128               # key chunks
    npair = nkc // 2               # full chunk pairs
    odd = nkc % 2 == 1
    nclean = max(0, min(npair, n_min // 256))  # pairs with no padded keys

    ctx_pools = []

    def pool(name, bufs, space="SBUF"):
        p = tc.tile_pool(name=name, bufs=bufs, space=space)
        ctx_pools.append(p)
        return p.__enter__()

    store = pool("store", 1)

    # ---- persistent tiles ----
    xqT = store.tile([128, KCH, S], BF16)
    xkvT = store.tile([128, KCH, skv], BF16)
    qT = store.tile([128, 3, S], QK_DT)
    kT = store.tile([128, 3, skv], QK_DT)
    vst = store.tile([128, nkc, HC, VW], PV_DT)
    att = store.tile([128, VC // 128, S], ATT_DT)
    wq = store.tile([128, KCH, VC], QK_DT)
    wk = store.tile([128, KCH, VC], QK_DT)
    wv = store.tile([128, KCH, VC], BF16)
    wp = store.tile([128, VC // 128, E], ATT_DT)
    bq = store.tile([128, 3], F32)
    bk = store.tile([128, 3], F32)
    mb = store.tile([128, nkc], F32)
    ones_row = store.tile([1, D], BF16)

    # weights: one dma each (partition-major in DRAM); k/v weights go on
    # the scalar HWDGE queue so they land before the first projections,
    # q/p weights take the SWDGE (gpsimd) queue.
    nc.scalar.dma_start(wk[:].rearrange("p a b -> p (a b)"), wk_d.ap())
    nc.scalar.dma_start(wv[:].rearrange("p a b -> p (a b)"), wv_d.ap())
    nc.gpsimd.dma_start(bk[:], bk_d.ap().rearrange("(c p) -> p c", p=128))
    nc.gpsimd.dma_start(bq[:], bq_d.ap().rearrange("(c p) -> p c", p=128))
    nc.gpsimd.dma_start(mb[:], mb_d.ap().rearrange("(c p) -> p c", p=128))
    nc.gpsimd.dma_start(wq[:].rearrange("p a b -> p (a b)"), wq_d.ap())
    nc.gpsimd.dma_start(wp[:].rearrange("p a b -> p (a b)"), wp_d.ap())

    nc.vector.memset(ones_row[:], 1.0 / ATT_SCALE)
    with nc.allow_low_precision(reason="ones column exact in low precision"):
        nc.vector.memset(vst[:, :, :, D : D + 1], 1.0)

    # ---- x transposes via DMA xbar: kv first, then q block 0, then rest.
    # Alternate between the two HWDGE queues so dispatch overheads overlap.
    def xpose(eng, out, in_):
        eng.dma_start_transpose(out, in_)

    q2 = [nc.sync, nc.scalar]
    for fb in range(KCH):
        xpose(q2[fb % 2], xkvT[:, fb, :],
              xkv_d.ap()[:, fb * 128 : (fb + 1) * 128])
    for fb in range(KCH):
        xpose(q2[fb % 2], xqT[:, fb, :],
              xq_d.ap()[:, fb * 128 : (fb + 1) * 128])

    lp = nc.allow_low_precision(reason="bf16/fp8 attention pipeline by design")
    lp.__enter__()

    # ---- phase A: k/v projections ----
    with (
        tc.tile_pool(name="kp", bufs=2, space="PSUM") as kp_p,
        tc.tile_pool(name="va", bufs=2, space="PSUM") as va_p,
    ):
        def vchunk(c):
            va = va_p.tile([128, 512], F32, tag="va")
            for k in range(KCH):
                nc.tensor.matmul(
                    va[:, 0:VC], xkvT[:, k, c * 128 : (c + 1) * 128],
                    wv[:, k, :],
                    start=(k == 0), stop=(k == KCH - 1),
                )
            nc.vector.tensor_copy(
                vst[:, c, :, 0:D],
                va[:, 0:VC].rearrange("p (h d) -> p h d", h=HC),
            )

        kblocks = [(c0, min(512, skv - c0)) for c0 in range(0, skv, 512)]
        vi = 0
        for bi, (c0, cw) in enumerate(kblocks):
            for m in range(3):
                kp = kp_p.tile([128, 512], F32, tag="kp")
                for k in range(KCH):
                    nc.tensor.matmul(
                        kp[:, :cw], wk[:, k, m * 128 : (m + 1) * 128],
                        xkvT[:, k, c0 : c0 + cw],
                        start=(k == 0), stop=(k == KCH - 1),
                    )
                nc.vector.tensor_scalar_add(
                    kT[:, m, c0 : c0 + cw], kp[:, :cw], bk[:, m : m + 1]
                )
            nv = 4 if bi == 0 else 3
            for _ in range(nv):
                if vi < nkc:
                    vchunk(vi); vi += 1
        while vi < nkc:
            vchunk(vi); vi += 1

    # ---- phase B: per q-block: q-proj, scores+exp+pv, norm; out-proj is
    # emitted one block late so PE keeps feeding ACT across qb boundaries.
    st_p = pool("st", 2, space="PSUM")       # [128,1024] = 2 banks each
    pv_p = pool("pv", 3, space="PSUM")       # pv + q-proj tiles
    bc_p = pool("bc", 1, space="PSUM")       # den broadcast
    pt_p = pool("pt", 3)
    se_p = pool("se", 3)
    rb_p = pool("rb", 3)
    ys_p = pool("ys", 2)

    def out_proj(qb):
        for sc in range(4):
            sg = qb * 4 + sc
            ssl = slice(sg * 128, (sg + 1) * 128)
            ya = st_p.tile([128, 1024], F32, tag="st", name=f"ya{sg}")
            for n0, nw in ((0, 512), (512, 256)):
                if OP_FP8:
                    nc.tensor.matmul(
                        ya[:, n0 : n0 + nw], att[:, 0:2, ssl],
                        wp[:, 0:2, n0 : n0 + nw],
                        start=True, stop=False, perf_mode=DR,
                    )
                    nc.tensor.matmul(
                        ya[:, n0 : n0 + nw], att[:, 2, ssl],
                        wp[:, 2, n0 : n0 + nw],
                        start=False, stop=True,
                    )
                else:
                    for t in range(VC // 128):
                        nc.tensor.matmul(
                            ya[:, n0 : n0 + nw], att[:, t, ssl],
                            wp[:, t, n0 : n0 + nw],
                            start=(t == 0), stop=(t == VC // 128 - 1),
                        )
            ys = ys_p.tile([128, E], BF16, tag="ys")
            nc.vector.tensor_copy(ys[:], ya[:, 0:E])
            nc.sync.dma_start(y_d.ap()[ssl, :], ys[:])

    deferred_norms = []
    for qb in range(NQB):
        qs = slice(qb * 512, (qb + 1) * 512)
        # q projection for this block
        for m in range(3):
            qp = pv_p.tile([128, 512], F32, tag="pv", name=f"qp{qb}_{m}")
            for k in range(KCH):
                nc.tensor.matmul(
                    qp[:], wq[:, k, m * 128 : (m + 1) * 128],
                    xqT[:, k, qs],
                    start=(k == 0), stop=(k == KCH - 1),
                )
            nc.vector.tensor_scalar_add(
                qT[:, m, qs], qp[:], bq[:, m : m + 1]
            )

        # two heads run interleaved per pass (one st slot each) so the
        # PE->ACT->PE handshake latency of one head hides under the other.
        nsteps = npair + (1 if odd else 0)

        def emit_pv(h, pv, step, pt):
            first = step == 0
            last = step == nsteps - 1
            if step == npair:  # odd tail chunk
                nc.tensor.matmul(
                    pv[0:VW, :], vst[:, nkc - 1, h, :], pt[:, 0, :],
                    start=first, stop=last,
                )
            elif PV_FP8:
                nc.tensor.matmul(
                    pv[0:VW, :], vst[:, 2 * step : 2 * step + 2, h, :],
                    pt[:], start=first, stop=last, perf_mode=DR,
                )
            else:
                for sub in range(2):
                    nc.tensor.matmul(
                        pv[0:VW, :], vst[:, 2 * step + sub, h, :],
                        pt[:, sub, :],
                        start=first and sub == 0, stop=last and sub == 1,
                    )

        def norm(h, pv, qs):
            # att[h] = pv[0:D] * (ATT_SCALE / pv[D])
            se = se_p.tile([1, 512], BF16, tag="se")
            nc.vector.tensor_copy(se[:], pv[D : D + 1, :])
            bc = bc_p.tile([128, 512], F32, tag="bc")
            nc.tensor.matmul(bc[0:D, :], ones_row[:], se[:],
                             start=True, stop=True)
            rb = rb_p.tile([D, 512], F32R, tag="rb")
            nc.vector.reciprocal(rb[:], bc[0:D, :])
            nc.vector.tensor_tensor(
                att[(h % 2) * D : (h % 2) * D + D, h // 2, qs],
                pv[0:D, :], rb[:], op=AluOpType.mult,
            )

        for hp in range(HC // 2):
            heads = (2 * hp, 2 * hp + 1)
            pvt = {h: pv_p.tile([128, 512], F32, tag="pv", name=f"pv{qb}_{h}")
                   for h in heads}
            qsl = {h: _score_slices(qT, h, qs)[0] for h in heads}
            pending = {h: None for h in heads}
            for step in range(nsteps):
                for h in heads:
                    st = st_p.tile([128, 1024], F32, tag="st")
                    subs = 1 if step == npair else 2
                    for sub in range(subs):
                        kc = 2 * step + sub
                        ksl, isdr = _score_slices(
                            kT, h, slice(kc * 128, (kc + 1) * 128))
                        nc.tensor.matmul(
                            st[:, sub * 512 : (sub + 1) * 512], ksl, qsl[h],
                            start=True, stop=True,
                            perf_mode=DR if isdr else None,
                        )
                    if pending[h] is not None:
                        emit_pv(h, pvt[h], *pending[h])
                    elif deferred_norms:
                        norm(*deferred_norms.pop(0))
                    pt = pt_p.tile([128, 2, 512], PV_DT, tag="pt")
                    if step < nclean:
                        nc.scalar.activation(
                            pt[:].rearrange("p a b -> p (a b)"), st[:],
                            Act.Exp, bias=0.0, scale=0.125,
                        )
                    else:
                        for sub in range(subs):
                            kc = 2 * step + sub
                            nc.scalar.activation(
                                pt[:, sub, :],
                                st[:, sub * 512 : (sub + 1) * 512],
                                Act.Exp, bias=mb[:, kc : kc + 1], scale=0.125,
                            )
                    pending[h] = (step, pt)
            for h in heads:
                emit_pv(h, pvt[h], *pending[h])
                deferred_norms.append((h, pvt[h], qs))
            # previous q-block's projection, once its last norms are done
            if hp == 1 and qb > 0:
                out_proj(qb - 1)
    while deferred_norms:
        norm(*deferred_norms.pop(0))
    out_proj(NQB - 1)

    if dbg is not None:
        nc.sync.dma_start(dbg["qT"].ap(), qT[:].rearrange("p a b -> p (a b)"))
        nc.sync.dma_start(dbg["kT"].ap(), kT[:].rearrange("p a b -> p (a b)"))
        nc.sync.dma_start(dbg["vst"].ap(), vst[:].rearrange("p a b c -> p (a b c)"))
        nc.sync.dma_start(dbg["att"].ap(), att[:].rearrange("p a b -> p (a b)"))
        nc.sync.dma_start(dbg["xqT"].ap(), xqT[:].rearrange("p a b -> p (a b)"))
        nc.sync.dma_start(dbg["xkvT"].ap(), xkvT[:].rearrange("p a b -> p (a b)"))

    lp.__exit__(None, None, None)
    for p in reversed(ctx_pools):
        p.__exit__(None, None, None)


def _reorder_cols():
    """Column permutation (within a core's 384 q/k columns) for SC_FP8."""
    if not SC_FP8:
        return np.arange(VC)
    perm = []
    for h in range(4):
        perm.extend(range(h * 64, h * 64 + 32))        # m0: lo dims h0..h3
    for h in range(4):
        perm.extend(range(h * 64 + 32, h * 64 + 64))   # m1: hi dims h0..h3
    for h in range(4, 6):
        perm.extend(range(h * 64, h * 64 + 64))        # m2: h4, h5
    return np.array(perm)


def _wtile(w, dt):
    """[E, cols] -> partition-major [128, KCH*cols] in dtype dt."""
    cols = w.shape[1]
    return np.ascontiguousarray(
        w.reshape(KCH, 128, cols).transpose(1, 0, 2).reshape(128, KCH * cols)
    ).astype(np_dt(dt))


def make_core_inputs(x, mask, Wqkv, bqkv, Wproj):
    x = np.asarray(x, dtype=np.float32)
    mask = np.asarray(mask)
    Wqkv = np.asarray(Wqkv, dtype=np.float32)
    bqkv = np.asarray(bqkv, dtype=np.float32)
    Wproj = np.asarray(Wproj, dtype=np.float32)

    idxs = [np.nonzero(mask[b, 0, 0, :] != 0)[0] for b in range(B)]
    ns = [len(ix) for ix in idxs]
    skv = max(128, ((max(ns) + 127) // 128) * 128)
    n_min = min(ns)

    perm = _reorder_cols()

    in_maps = []
    for c in range(8):
        b = c // 2
        h0 = (c % 2) * HC
        ix, n = idxs[b], ns[b]
        xkv = np.zeros((skv, E), dtype=ml_dtypes.bfloat16)
        xkv[:n] = x[b][ix].astype(ml_dtypes.bfloat16)
        mbv = np.zeros(skv, dtype=np.float32)
        mbv[n:] = MASK_NEG
        wq_c = Wqkv[:, h0 * D : (h0 + HC) * D][:, perm]
        wk_c = Wqkv[:, E + h0 * D : E + (h0 + HC) * D][:, perm]
        bq_c = bqkv[h0 * D : (h0 + HC) * D][perm]
        bk_c = bqkv[E + h0 * D : E + (h0 + HC) * D][perm]
        wv_c = Wqkv[:, 2 * E + h0 * D : 2 * E + (h0 + HC) * D]
        # wp rows are the core's 384 att dims -> [128, 3*E] partition-major
        wp_c = (Wproj[h0 * D : (h0 + HC) * D, :] / ATT_SCALE)
        wp_t = np.ascontiguousarray(
            wp_c.reshape(VC // 128, 128, E).transpose(1, 0, 2)
            .reshape(128, (VC // 128) * E)
        ).astype(np_dt(ATT_DT))
        in_maps.append({
            "xq": np.ascontiguousarray(x[b].astype(ml_dtypes.bfloat16)),
            "xkv": np.ascontiguousarray(xkv),
            "wq": _wtile(wq_c, QK_DT),
            "wk": _wtile(wk_c, QK_DT),
            "wv": _wtile(wv_c, BF16),
            "wp": wp_t,
            "bq": np.ascontiguousarray(bq_c),
            "bk": np.ascontiguousarray(bk_c),
            "mb": np.ascontiguousarray(mbv),
        })
    return in_maps, skv, n_min


def run(x, mask, Wqkv, bqkv, Wproj, bproj, trace=False, trace_cores=None):
    bqkv_np = np.asarray(bqkv, dtype=np.float32)
    Wproj_np = np.asarray(Wproj, dtype=np.float32)
    bproj_np = np.asarray(bproj, dtype=np.float32)
    in_maps, skv, n_min = make_core_inputs(x, mask, Wqkv, bqkv_np, Wproj_np)
    _BUILD["skv"], _BUILD["n_min"] = skv, n_min

    nc = build_program(skv, n_min)
    try:
        res = run_bass_kernel_spmd(
            nc, in_maps, core_ids=list(range(8)), trace=trace,
            trace_cores=trace_cores,
        )
    except Exception:
        # transient device wedge — one retry is usually enough
        res = run_bass_kernel_spmd(
            nc, in_maps, core_ids=list(range(8)), trace=trace,
            trace_cores=trace_cores,
        )
    parts = [res.results[c]["y"].astype(np.float32) for c in range(8)]

    # host-folded bias: v-bias passes through softmax (weights sum to 1),
    # so y += bv @ Wproj + bproj, applied once per batch row.
    bv = bqkv_np[2 * E : 3 * E]
    bias_row = bv @ Wproj_np + bproj_np
    y = np.stack(
        [parts[2 * b] + parts[2 * b + 1] + bias_row for b in range(B)]
    ).astype(np.float32)
    return y, res


def kernel(x, mask, Wqkv, bqkv, Wproj, bproj):
    y, _ = run(x, mask, Wqkv, bqkv, Wproj, bproj, trace=False)
    return y


# revision 43
# speedup vs baseline: 1.5948x; 1.0935x over previous
"""Trainium2 Bass kernel for CodeAttention (B=4, S=2048, E=768, H=12).

Sharding: 8 cores = 4 batches x 2 head-groups (6 heads each); each core
computes a partial projection output for its batch, host sums the two
partials and adds the (host-folded) v-bias/proj-bias row.

Key optimizations over the naive formulation:
 - masked keys (mask==0, ~50%) are compacted away on the host: K/V are
   computed only for unmasked key positions (exact: reference's -1e9
   additive mask makes masked keys contribute exactly 0).
 - x / weights travel as bf16; x is transposed by the DMA xbar
   (dma_start_transpose) instead of PE identity-matmuls.
 - optional fp8e4 DoubleRow matmuls (2 contraction groups per pass) for
   the score, attn*V and output-projection stages.
"""

import sys

if "/opt/trn_rl_repo" not in sys.path:
    sys.path.insert(0, "/opt/trn_rl_repo")

import ml_dtypes
import numpy as np

import concourse.bass as bass  # noqa: F401
import concourse.mybir as mybir
import concourse.tile as tile
from concourse import bacc
from concourse.alu_op_type import AluOpType
from concourse.bass_utils import run_bass_kernel_spmd

F32 = mybir.dt.float32
F32R = mybir.dt.float32r
BF16 = mybir.dt.bfloat16
FP8 = mybir.dt.float8e4
Act = mybir.ActivationFunctionType
DR = mybir.MatmulPerfMode.DoubleRow

B, S, E, H, D = 4, 2048, 768, 12, 64
HC = 6                    # heads per core
KCH = E // 128            # x-feature chunks = 6
VC = HC * D               # v columns per core = 384
VW = D + 1                # v width incl. ones column = 65
NQB = S // 512            # q blocks = 4
MASK_NEG = -50.0

# fp8 DoubleRow stage flags (host + device layouts derive from these)
SC_FP8 = False            # scores q@k in fp8e4 (+ lo/hi W-column layout)
PV_FP8 = False            # attn*V in fp8e4 (pt + vst)
OP_FP8 = False            # out-proj in fp8e4 (att scaled x16, wp/16)

QK_DT = FP8 if SC_FP8 else BF16
NMB = 4 if SC_FP8 else 3          # q/k m-blocks (4th holds hi-dims when fp8)
QC = NMB * 128                    # q/k columns incl. padding
PV_DT = FP8 if PV_FP8 else BF16
VWP = 128 if PV_FP8 else VW   # padded v width (DR ldweights wants 32/64/128)
# constant shift inside exp keeps fp8 pt in range (max logit ~5.5 would
# overflow e4m3's 240); cancels exactly in the softmax ratio.
EXP_BIAS = -4.0 if PV_FP8 else 0.0
ATT_DT = FP8 if OP_FP8 else BF16
ATT_SCALE = 16.0 if OP_FP8 else 1.0

# defaults for build_program() without a preceding run() (TimelineSim in
# test.py); overwritten by run() with the values for the actual inputs.
_BUILD = {"skv": 1152, "n_min": 986}


def np_dt(dt):
    return mybir.dt.np(dt)


def build_program(skv=None, n_min=None, debug=False):
    skv = skv if skv is not None else _BUILD["skv"]
    n_min = n_min if n_min is not None else _BUILD["n_min"]
    nc = bacc.Bacc("TRN2", target_bir_lowering=False, debug=False, num_devices=8)

    xq_d = nc.dram_tensor("xq", [S, E], BF16, kind="ExternalInput")
    xkv_d = nc.dram_tensor("xkv", [skv, E], BF16, kind="ExternalInput")
    # weights pre-transposed on host to partition-major [128, KCH*cols]
    wq_d = nc.dram_tensor("wq", [128, KCH * QC], QK_DT, kind="ExternalInput")
    wk_d = nc.dram_tensor("wk", [128, KCH * QC], QK_DT, kind="ExternalInput")
    wv_d = nc.dram_tensor("wv", [128, KCH * VC], BF16, kind="ExternalInput")
    wp_d = nc.dram_tensor("wp", [128, (VC // 128) * E], ATT_DT, kind="ExternalInput")
    bq_d = nc.dram_tensor("bq", [QC], F32, kind="ExternalInput")
    bk_d = nc.dram_tensor("bk", [QC], F32, kind="ExternalInput")
    rc_d = nc.dram_tensor("rc", [256 * ((skv // 128 + 1) // 2)], F32, kind="ExternalInput")
    y_d = nc.dram_tensor("y", [S, E], BF16, kind="ExternalOutput")
    dbg = None
    if debug:
        dbg = {
            "qT": nc.dram_tensor("dbg_qT", [128, NMB * S], QK_DT, kind="ExternalOutput"),
            "kT": nc.dram_tensor("dbg_kT", [128, NMB * skv], QK_DT, kind="ExternalOutput"),
            "vst": nc.dram_tensor("dbg_vst", [128, ((skv // 128 + 1) // 2) * HC * 2 * VWP], PV_DT, kind="ExternalOutput"),
            "att": nc.dram_tensor("dbg_att", [128, 3 * S], ATT_DT, kind="ExternalOutput"),
            "xqT": nc.dram_tensor("dbg_xqT", [128, KCH * S], BF16, kind="ExternalOutput"),
            "xkvT": nc.dram_tensor("dbg_xkvT", [128, KCH * skv], BF16, kind="ExternalOutput"),
        }

    with tile.TileContext(nc) as tc:
        _emit(nc, tc, skv, n_min,
              xq_d, xkv_d, wq_d, wk_d, wv_d, wp_d, bq_d, bk_d, rc_d, y_d, dbg)
    nc.compile()
    return nc


def _score_slices(t, h, kc_or_q):
    """(sliced AP, DR?) for head h in the qT/kT layout."""
    if SC_FP8:
        base = 32 * (h % 3)
        mp = 2 * (h // 3)
        return t[base : base + 32, mp : mp + 2, kc_or_q], True
    return t[64 * (h % 2) : 64 * (h % 2) + 64, h // 2, kc_or_q], False


def _emit(nc, tc, skv, n_min,
          xq_d, xkv_d, wq_d, wk_d, wv_d, wp_d, bq_d, bk_d, rc_d, y_d, dbg=None):
    nkc = skv // 128               # key chunks
    npair = nkc // 2               # full chunk pairs
    odd = nkc % 2 == 1

    ctx_pools = []

    def pool(name, bufs, space="SBUF"):
        p = tc.tile_pool(name=name, bufs=bufs, space=space)
        ctx_pools.append(p)
        return p.__enter__()

    store = pool("store", 1)

    # ---- persistent tiles ----
    xqT0 = store.tile([128, KCH, 512], BF16)    # q-block 0 (early)
    xqTr = store.tile([128, KCH, S - 512], BF16)
    xkvT = store.tile([128, KCH, skv], BF16)
    qT = store.tile([128, NMB, S], QK_DT)
    kT = store.tile([128, NMB, skv], QK_DT)
    npv = npair + (1 if odd else 0)
    vst = store.tile([128, npv, HC, 2, VWP], PV_DT)
    att = store.tile([128, VC // 128, S], ATT_DT)
    wq = store.tile([128, KCH, QC], QK_DT)
    wk = store.tile([128, KCH, QC], QK_DT)
    wv = store.tile([128, KCH, VC], BF16)
    wp = store.tile([128, VC // 128, E], ATT_DT)
    bq = store.tile([128, NMB], F32)
    bk = store.tile([128, NMB], F32)
    rc = store.tile([128, 2 * (npair + (1 if odd else 0))], F32)
    ones_row = store.tile([1, D], BF16)
    ebias = store.tile([128, 1], F32)

    # DMAs are emitted in need-order: the tile scheduler's DMA-lane
    # semaphore ticks follow emission order, so anything emitted early
    # falsely gates later DMAs on the same lane slots.
    nc.scalar.dma_start(wk[:].rearrange("p a b -> p (a b)"), wk_d.ap())
    nc.scalar.dma_start(wv[:].rearrange("p a b -> p (a b)"), wv_d.ap())

    nc.vector.memset(ones_row[:], 1.0 / ATT_SCALE)
    nc.vector.memset(ebias[:], EXP_BIAS)

    # ---- x transposes via DMA xbar: one instruction produces all KCH
    # feature blocks (out 3D [128, KCH, rows], test_dma_transpose2
    # pattern). xq is split so q-block 0 lands early; each piece writes a
    # whole tile (offset writes into one tile are broken on hw).
    nc.gpsimd.dma_start(wq[:].rearrange("p a b -> p (a b)"), wq_d.ap())
    nc.gpsimd.dma_start(bk[:], bk_d.ap().rearrange("(c p) -> p c", p=128))
    nc.sync.dma_start_transpose(xkvT[:], xkv_d.ap())
    nc.sync.dma_start_transpose(xqT0[:], xq_d.ap()[0:512, :])
    nc.sync.dma_start_transpose(xqTr[:], xq_d.ap()[512:S, :])
    nc.gpsimd.dma_start(bq[:], bq_d.ap().rearrange("(c p) -> p c", p=128))
    nc.gpsimd.dma_start(rc[:], rc_d.ap().rearrange("(c p) -> p c", p=128))
    nc.gpsimd.dma_start(wp[:].rearrange("p a b -> p (a b)"), wp_d.ap())

    # padded keys are excluded by zeroing their "ones" column: they then
    # contribute exactly 0 to both the attn*V numerator and the softmax
    # denominator, so no mask bias is needed in the exps at all.
    with nc.allow_low_precision(reason="0/1 indicator exact in low precision"):
        if VWP > VW:
            nc.gpsimd.memset(vst[:, :, :, :, VW:VWP], 0.0)
        for h in range(HC):
            nc.vector.tensor_copy(
                vst[:, :, h, :, D : D + 1],
                rc[:].rearrange("p (a b) -> p a b", b=2),
            )

    lp = nc.allow_low_precision(reason="bf16/fp8 attention pipeline by design")
    lp.__enter__()

    # ---- pools (k/v/q projections share the attention-stream pools so
    # projection work can interleave with scores as filler) ----
    st_p = pool("st", 2, space="PSUM")       # [128,1024] = 2 banks each
    pv_p = pool("pv", 3, space="PSUM")       # pv accumulators
    bc_p = pool("bc", 1, space="PSUM")       # den broadcast + q-proj
    pt_p = pool("pt", 3)
    se_p = pool("se", 3)
    rb_p = pool("rb", 3)
    ys_p = pool("ys", 2)

    def vchunk(c):
        va = st_p.tile([128, 1024], F32, tag="st", name=f"va{c}")
        for k in range(KCH):
            nc.tensor.matmul(
                va[:, 0:VC], xkvT[:, k, c * 128 : (c + 1) * 128],
                wv[:, k, :],
                start=(k == 0), stop=(k == KCH - 1),
            )
        nc.vector.tensor_copy(
            vst[:, c // 2, :, c % 2, 0:D],
            va[:, 0:VC].rearrange("p (h d) -> p h d", h=HC),
        )

    kblocks = [(c0, min(512, skv - c0)) for c0 in range(0, skv, 512)]

    def kproj(bi, m):
        c0, cw = kblocks[bi]
        kp = st_p.tile([128, 1024], F32, tag="st", name=f"kp{bi}_{m}")
        for k in range(KCH):
            nc.tensor.matmul(
                kp[:, :cw], wk[:, k, m * 128 : (m + 1) * 128],
                xkvT[:, k, c0 : c0 + cw],
                start=(k == 0), stop=(k == KCH - 1),
            )
        nc.vector.tensor_scalar_add(
            kT[:, m, c0 : c0 + cw], kp[:, :cw], bk[:, m : m + 1]
        )

    def out_proj_sc(qb, sc):
        sg = qb * 4 + sc
        ssl = slice(sg * 128, (sg + 1) * 128)
        ya = st_p.tile([128, 1024], F32, tag="st", name=f"ya{sg}")
        for n0, nw in ((0, 512), (512, 256)):
            if OP_FP8:
                nc.tensor.matmul(
                    ya[:, n0 : n0 + nw], att[:, 0:2, ssl],
                    wp[:, 0:2, n0 : n0 + nw],
                    start=True, stop=False, perf_mode=DR,
                )
                nc.tensor.matmul(
                    ya[:, n0 : n0 + nw], att[:, 2, ssl],
                    wp[:, 2, n0 : n0 + nw],
                    start=False, stop=True,
                )
            else:
                for t in range(VC // 128):
                    nc.tensor.matmul(
                        ya[:, n0 : n0 + nw], att[:, t, ssl],
                        wp[:, t, n0 : n0 + nw],
                        start=(t == 0), stop=(t == VC // 128 - 1),
                    )
        ys = ys_p.tile([128, E], BF16, tag="ys")
        if qb == NQB - 1 and sc % 2 == 0:
            nc.scalar.activation(ys[:], ya[:, 0:E], Act.Identity)
        else:
            nc.vector.tensor_copy(ys[:], ya[:, 0:E])
        nc.sync.dma_start(y_d.ap()[ssl, :], ys[:])

    def qproj_m(qb, m):
        # bc pool: its previous tile's readers are always already emitted
        qp = bc_p.tile([128, 512], F32, tag="bc", name=f"qp{qb}_{m}")
        qs = slice(qb * 512, (qb + 1) * 512)
        for k in range(KCH):
            xsl = (xqT0[:, k, :] if qb == 0
                   else xqTr[:, k, (qb - 1) * 512 : qb * 512])
            nc.tensor.matmul(
                qp[:], wq[:, k, m * 128 : (m + 1) * 128], xsl,
                start=(k == 0), stop=(k == KCH - 1),
            )
        nc.vector.tensor_scalar_add(qT[:, m, qs], qp[:], bq[:, m : m + 1])

    # minimal prologue: the first head-pair (h0,h1) only needs q/k m-blocks
    # 0-1, kT block 0 and the first two v chunks; the rest of the k/v/q
    # projections drain as fillers inside the first q-block's step loop.
    kproj(0, 0)
    kproj(0, 1)
    vchunk(0)
    vchunk(1)
    qproj_m(0, 0)
    qproj_m(0, 1)

    deferred_norms = []
    for qb in range(NQB):
        qs = slice(qb * 512, (qb + 1) * 512)
        # PE filler work drained a few items per step so ACT never starves.
        if qb == 0:
            fillers = [(lambda c=c: vchunk(c)) for c in (2, 3)]
            fillers += [(lambda bi=bi, m=m: kproj(bi, m))
                        for bi in range(1, len(kblocks)) for m in (0, 1)]
            fillers += [(lambda c=c: vchunk(c)) for c in range(4, nkc)]
            fillers += [(lambda m=m: qproj_m(0, m)) for m in range(2, NMB)]
            fillers += [(lambda bi=bi, m=m: kproj(bi, m))
                        for bi in range(len(kblocks)) for m in range(2, NMB)]
            fillers += [(lambda m=m: qproj_m(1, m)) for m in range(NMB)]
            rate = 3
        else:
            fillers = []
            if qb + 1 < NQB:
                fillers += [(lambda qb=qb, m=m: qproj_m(qb + 1, m))
                            for m in range(NMB)]
            fillers += [(lambda qb=qb, sc=sc: out_proj_sc(qb - 1, sc))
                        for sc in range(4)]
            rate = 1
        nfill = 0

        # two heads run interleaved per pass (one st slot each) so the
        # PE->ACT->PE handshake latency of one head hides under the other.
        nsteps = npair + (1 if odd else 0)

        def emit_pv(h, pv, step, pt):
            first = step == 0
            last = step == nsteps - 1
            if step == npair:  # odd tail chunk
                nc.tensor.matmul(
                    pv[0:VWP, :], vst[:, npair, h, 0, :], pt[:, 0, :],
                    start=first, stop=last,
                )
            elif PV_FP8:
                nc.tensor.matmul(
                    pv[0:VWP, :], vst[:, step, h, 0:2, :],
                    pt[:], start=first, stop=last, perf_mode=DR,
                )
            else:
                for sub in range(2):
                    nc.tensor.matmul(
                        pv[0:VWP, :], vst[:, step, h, sub, :],
                        pt[:, sub, :],
                        start=first and sub == 0, stop=last and sub == 1,
                    )

        def norm(h, pv, qs):
            # att[h] = pv[0:D] * (ATT_SCALE / pv[D])
            se = se_p.tile([1, 512], BF16, tag="se")
            nc.vector.tensor_copy(se[:], pv[D : D + 1, :])
            bc = bc_p.tile([128, 512], F32, tag="bc")
            nc.tensor.matmul(bc[0:D, :], ones_row[:], se[:],
                             start=True, stop=True)
            rb = rb_p.tile([D, 512], F32R, tag="rb")
            nc.vector.reciprocal(rb[:], bc[0:D, :])
            nc.vector.tensor_tensor(
                att[(h % 2) * D : (h % 2) * D + D, h // 2, qs],
                pv[0:D, :], rb[:], op=AluOpType.mult,
            )

        for hp in range(HC // 2):
            heads = (2 * hp, 2 * hp + 1)
            pvt = {h: pv_p.tile([128, 512], F32, tag="pv", name=f"pv{qb}_{h}")
                   for h in heads}
            qsl = {h: _score_slices(qT, h, qs)[0] for h in heads}
            pending = {h: None for h in heads}
            for step in range(nsteps):
                for h in heads:
                    st = st_p.tile([128, 1024], F32, tag="st")
                    subs = 1 if step == npair else 2
                    for sub in range(subs):
                        kc = 2 * step + sub
                        ksl, isdr = _score_slices(
                            kT, h, slice(kc * 128, (kc + 1) * 128))
                        nc.tensor.matmul(
                            st[:, sub * 512 : (sub + 1) * 512], ksl, qsl[h],
                            start=True, stop=True,
                            perf_mode=DR if isdr else None,
                        )
                    if pending[h] is not None:
                        emit_pv(h, pvt[h], *pending[h])
                    elif deferred_norms:
                        norm(*deferred_norms.pop(0))
                    pt = pt_p.tile([128, 2, 512], PV_DT, tag="pt")
                    if subs == 2:
                        nc.scalar.activation(
                            pt[:].rearrange("p a b -> p (a b)"), st[:],
                            Act.Exp, bias=ebias[:], scale=0.125,
                        )
                    else:
                        nc.scalar.activation(
                            pt[:, 0, :], st[:, 0:512],
                            Act.Exp, bias=ebias[:], scale=0.125,
                        )
                    pending[h] = (step, pt)
                nfill += 1
                # gate out-proj fillers until the previous block's last
                # norms (popped during this block's first steps) are in
                for _ in range(rate):
                    if fillers and (qb == 0 or nfill >= 3 or len(fillers) > 4):
                        fillers.pop(0)()
            for h in heads:
                emit_pv(h, pvt[h], *pending[h])
                deferred_norms.append((h, pvt[h], qs))
        while fillers:
            fillers.pop(0)()
    while deferred_norms:
        norm(*deferred_norms.pop(0))
    for sc in range(4):
        out_proj_sc(NQB - 1, sc)

    if dbg is not None:
        nc.sync.dma_start(dbg["qT"].ap(), qT[:].rearrange("p a b -> p (a b)"))
        nc.sync.dma_start(dbg["kT"].ap(), kT[:].rearrange("p a b -> p (a b)"))
        nc.sync.dma_start(dbg["vst"].ap(), vst[:].rearrange("p a b c d -> p (a b c d)"))
        nc.sync.dma_start(dbg["att"].ap(), att[:].rearrange("p a b -> p (a b)"))
        nc.sync.dma_start(dbg["xqT"].ap().rearrange("p (a b) -> p a b", a=KCH)[:, :, 0:512], xqT0[:])
        nc.sync.dma_start(dbg["xqT"].ap().rearrange("p (a b) -> p a b", a=KCH)[:, :, 512:S], xqTr[:])
        nc.sync.dma_start(dbg["xkvT"].ap(), xkvT[:].rearrange("p a b -> p (a b)"))

    lp.__exit__(None, None, None)
    for p in reversed(ctx_pools):
        p.__exit__(None, None, None)


def _qk_layout(w, b):
    """[E,384] weights / [384] bias -> SC_FP8 4-m-block layout ([E,QC]/[QC]):
    m(2g) = lo dims of heads 3g..3g+2 at col-bases 0/32/64, m(2g+1) = hi."""
    if not SC_FP8:
        return w, b
    wn = np.zeros((E, QC), np.float32)
    bn = np.zeros(QC, np.float32)
    for h in range(HC):
        g, i = h // 3, h % 3
        lo = (2 * g) * 128 + i * 32
        hi = (2 * g + 1) * 128 + i * 32
        wn[:, lo : lo + 32] = w[:, h * 64 : h * 64 + 32]
        wn[:, hi : hi + 32] = w[:, h * 64 + 32 : h * 64 + 64]
        bn[lo : lo + 32] = b[h * 64 : h * 64 + 32]
        bn[hi : hi + 32] = b[h * 64 + 32 : h * 64 + 64]
    return wn, bn


def _wtile(w, dt):
    """[E, cols] -> partition-major [128, KCH*cols] in dtype dt."""
    cols = w.shape[1]
    return np.ascontiguousarray(
        w.reshape(KCH, 128, cols).transpose(1, 0, 2).reshape(128, KCH * cols)
    ).astype(np_dt(dt))


def make_core_inputs(x, mask, Wqkv, bqkv, Wproj):
    x = np.asarray(x, dtype=np.float32)
    mask = np.asarray(mask)
    Wqkv = np.asarray(Wqkv, dtype=np.float32)
    bqkv = np.asarray(bqkv, dtype=np.float32)
    Wproj = np.asarray(Wproj, dtype=np.float32)

    idxs = [np.nonzero(mask[b, 0, 0, :] != 0)[0] for b in range(B)]
    ns = [len(ix) for ix in idxs]
    skv = max(128, ((max(ns) + 127) // 128) * 128)
    n_min = min(ns)

    in_maps = []
    for c in range(8):
        b = c // 2
        h0 = (c % 2) * HC
        ix, n = idxs[b], ns[b]
        xkv = np.zeros((skv, E), dtype=ml_dtypes.bfloat16)
        xkv[:n] = x[b][ix].astype(ml_dtypes.bfloat16)
        nkc_i = skv // 128
        npv_i = (nkc_i + 1) // 2
        rcv = np.zeros(npv_i * 256, dtype=np.float32)
        rcv[:n] = 1.0
        wq_c, bq_c = _qk_layout(Wqkv[:, h0 * D : (h0 + HC) * D],
                                bqkv[h0 * D : (h0 + HC) * D])
        wk_c, bk_c = _qk_layout(Wqkv[:, E + h0 * D : E + (h0 + HC) * D],
                                bqkv[E + h0 * D : E + (h0 + HC) * D])
        wv_c = Wqkv[:, 2 * E + h0 * D : 2 * E + (h0 + HC) * D]
        # wp rows are the core's 384 att dims -> [128, 3*E] partition-major
        wp_c = (Wproj[h0 * D : (h0 + HC) * D, :] / ATT_SCALE)
        wp_t = np.ascontiguousarray(
            wp_c.reshape(VC // 128, 128, E).transpose(1, 0, 2)
            .reshape(128, (VC // 128) * E)
        ).astype(np_dt(ATT_DT))
        in_maps.append({
            "xq": np.ascontiguousarray(x[b].astype(ml_dtypes.bfloat16)),
            "xkv": np.ascontiguousarray(xkv),
            "wq": _wtile(wq_c, QK_DT),
            "wk": _wtile(wk_c, QK_DT),
            "wv": _wtile(wv_c, BF16),
            "wp": wp_t,
            "bq": np.ascontiguousarray(bq_c.astype(np.float32)),
            "bk": np.ascontiguousarray(bk_c.astype(np.float32)),
            "rc": np.ascontiguousarray(rcv),
        })
    return in_maps, skv, n_min


def run(x, mask, Wqkv, bqkv, Wproj, bproj, trace=False, trace_cores=None):
    bqkv_np = np.asarray(bqkv, dtype=np.float32)
    Wproj_np = np.asarray(Wproj, dtype=np.float32)
    bproj_np = np.asarray(bproj, dtype=np.float32)
    in_maps, skv, n_min = make_core_inputs(x, mask, Wqkv, bqkv_np, Wproj_np)
    _BUILD["skv"], _BUILD["n_min"] = skv, n_min

    nc = build_program(skv, n_min)
    try:
        res = run_bass_kernel_spmd(
            nc, in_maps, core_ids=list(range(8)), trace=trace,
            trace_cores=trace_cores,
        )
    except Exception:
        # transient device wedge — one retry is usually enough
        res = run_bass_kernel_spmd(
            nc, in_maps, core_ids=list(range(8)), trace=trace,
            trace_cores=trace_cores,
        )
    parts = [res.results[c]["y"].astype(np.float32) for c in range(8)]

    # host-folded bias: v-bias passes through softmax (weights sum to 1),
    # so y += bv @ Wproj + bproj, applied once per batch row.
    bv = bqkv_np[2 * E : 3 * E]
    bias_row = bv @ Wproj_np + bproj_np
    y = np.stack(
        [parts[2 * b] + parts[2 * b + 1] + bias_row for b in range(B)]
    ).astype(np.float32)
    return y, res


def kernel(x, mask, Wqkv, bqkv, Wproj, bproj):
    y, _ = run(x, mask, Wqkv, bqkv, Wproj, bproj, trace=False)
    return y
